# revision 22
# baseline (speedup 1.0000x reference)
"""Multi-head attention (no softmax) Trainium2 Bass kernel, 8-core SPMD.

Reference computes out = ((X Wq + bq)(X Wk + bk)^T / 8)(X Wv + bv) Wo + bo
per head.  Since there is no softmax the per-head attention is linear:
    (Q_h K_h^T) V_h = Q_h (K_h^T V_h)
which collapses the 2048x2048 score matrix to a 64x64 per-head matrix.
Further folding (K_h^T V_h / 8) Wo_h into a data-dependent weight
W~ = stack_h(M_h Wo_h) makes the whole core computation:
    Q = X Wq_s,  KV = X [Wk_s|Wv_s],  MT_h = V_h^T K_h,
    W~_h = (MT_h^T / 8) Wo_h,  P = Q W~        (P is a partial of out)

Sharding: core c -> batch b = c // 4, head-group g = c % 4 (4 of 16 heads,
256 of 1024 projection columns).  No cross-device comms; the 4 partials per
batch are summed on the host (+ bo).

All matmul operands and DRAM traffic are bfloat16 (PE streams 1 row/cycle,
same as tf32, but SBUF/HBM bytes halve: ~10 MB/core total vs 21 MB at
fp32).  PSUM accumulation stays fp32; biases are added in fp32 on the
copy-backs; the output partial is written bf16 and the 4 partials per
batch are summed on the host in fp32 (measured end-to-end rel err ~4e-3
against the fp32 reference, vs a 2e-2 gate).

Schedule: dummy zero-matmuls warm the PE clock gate while the first DMAs
land; K|V projections chase the x^T token-block loads with the per-block
M accumulation (MT_h += V_h^T K_h) interleaved right behind each block
so M closes immediately after the last projection; W~ is the one sync
point (bracketed by Q^T projections, which are M-independent, to keep
the PE fed across its DVE handoffs); then Q^T and P = Q W~ interleave
per token block so the output DMA overlaps the remaining compute.

DMA notes (measured): each dma_start costs ~650ns of Sync-engine issue
time and per-core delivery tops out around 0.33 MB/us, so the input
rides as a handful of large transfers ordered to match the projection
loop's consumption (wkv halves interleaved with the first token blocks);
the PE streams chunk-by-chunk behind the DMA from ~10us and the kernel
is PE-bound from there (~61us of matmul cadence at 2.4 GHz).  Measured
best ~81.8us end-to-end per launch; device DVFS adds +-15% run-to-run.
"""

import numpy as np

import concourse.mybir as mybir
import concourse.tile as tile
from concourse import bacc
from concourse.bass_utils import run_bass_kernel_spmd

F32 = mybir.dt.float32
MM_DT = mybir.dt.bfloat16

B, L, DM = 2, 2048, 1024
QD = 256                 # per-core projection width (4 heads x 64)
HPC, HDIM = 4, 64
NCORES = 8
SCALE = 0.125            # 1 / sqrt(64)

DM_C = DM // 128         # 8 dmodel chunks
T_N = L // 128           # 16 token chunks (partition-sized)
T_F = L // 512           # 4 token chunks (free-dim sized)
QD_C = QD // 128         # 2 head-dim chunks / head-pairs
OUT_F = DM // 512        # 2 output free chunks


def build_program():
    nc = bacc.Bacc("TRN2", target_bir_lowering=False, debug=False)

    xT = nc.dram_tensor("xT", [DM, L], MM_DT, kind="ExternalInput")
    wq = nc.dram_tensor("wq", [DM, QD], MM_DT, kind="ExternalInput")
    wkv = nc.dram_tensor("wkv", [DM, 2 * QD], MM_DT, kind="ExternalInput")
    wo = nc.dram_tensor("wo", [QD, DM], MM_DT, kind="ExternalInput")
    bqt = nc.dram_tensor("bqt", [128, QD_C], F32, kind="ExternalInput")
    bkvd = nc.dram_tensor("bkvd", [1, 2 * QD], F32, kind="ExternalInput")
    pout = nc.dram_tensor("pout", [L, DM], MM_DT, kind="ExternalOutput")

    with tile.TileContext(nc) as tc:
        with (
            tc.tile_pool(name="persist", bufs=1) as pers,
            tc.tile_pool(name="pstage", bufs=4) as pstage,
            tc.tile_pool(name="psum", bufs=6, space="PSUM") as ps,
        ):
            # -------- PE warm-up: dummy matmuls with no DMA deps -----------
            # The HAM clock gate keeps an idle PE at 1.2 GHz; sustained
            # activity moves it to 2.4 GHz.  These zero-matmuls run while
            # the DMA descriptors are still being generated, so the first
            # real matmul executes at full rate.
            # Dummy matmuls bridge the PE from engine boot (~7us) until the
            # first projection's operands have landed (~12us); the HAM
            # up-clocks only after sustained activity, so the bridge also
            # pulls the 1.2->2.4 GHz transition several us earlier.
            N_WARM = 8
            warm_z = pers.tile([128, 512], MM_DT, tag="warmz")
            nc.vector.memzero(warm_z[:])
            warm_out = pers.tile([128, 512], F32, tag="warmout")
            pswarm = ps.tile([128, 512], F32, tag="ps", name="pswarm")
            for i in range(N_WARM):
                nc.tensor.matmul(
                    pswarm[:], lhsT=warm_z[:, 0:128], rhs=warm_z[:],
                    start=(i == 0), stop=(i == N_WARM - 1),
                )
            nc.vector.tensor_copy(warm_out[:], pswarm[:])

            # -------- wkv + x^T loads, interleaved per dmodel chunk ---------
            # The first K|V matmul needs only (wkv chunk 0, xt chunk 0 of
            # token block 0), so those bytes go on the queues first and the
            # PE starts a few us after the DMA engines spin up.  wq rides
            # after the first token block (its consumer P1 runs in the
            # tail); wo after all of x^T.
            wkv_sb = pers.tile([128, DM_C, 2 * QD], MM_DT, tag="wkv")
            wq_sb = pers.tile([128, DM_C, QD], MM_DT, tag="wq")
            xt_sb = pers.tile([128, DM_C, L], MM_DT, tag="xt")
            bq_sb = pers.tile([128, QD_C], F32, tag="bq")
            bkv_sb = pers.tile([1, 2 * QD], F32, tag="bkv1")
            bkv_bc = pers.tile([128, 2 * QD], F32, tag="bkvbc")
            # Each DMA instruction costs ~650ns of Sync-engine issue time,
            # so the input rides as a handful of large multi-chunk
            # transfers instead of per-chunk ones: all descriptors reach
            # the (16-way parallel) DMA hardware within ~3us of engine
            # boot and the transfer rate, not the issue rate, paces input.
            xT_r = xT.ap().rearrange("(c p) n -> p c n", p=128)
            wkv_r = wkv.ap().rearrange("(c p) n -> p c n", p=128)
            # wkv halves interleave with the first x^T tranches, and x^T
            # rides in arrival-ordered tranches sized to the P2 loop's
            # consumption (0.25 MB per 128-token block at ~0.13 MB/us,
            # under the ~0.2-0.33 MB/us delivery rate): the projection
            # loop's first matmuls start as soon as (wkv chunks 0-3, x
            # block 0) land and then stream chunk-by-chunk behind the DMA
            # (the open PSUM accumulation tolerates mid-group data waits),
            # so the PE never idles long enough for the HAM to down-clock.
            nc.sync.dma_start(wkv_sb[:, 0:4, :], wkv_r[:, 0:4, :])
            nc.sync.dma_start(xt_sb[:, :, 0:128], xT_r[:, :, 0:128])
            nc.sync.dma_start(wkv_sb[:, 4:8, :], wkv_r[:, 4:8, :])
            nc.sync.dma_start(bkv_sb[:], bkvd.ap())
            nc.gpsimd.partition_broadcast(bkv_bc[:], bkv_sb[:])
            # 2-3 block tranches: data becomes visible (dependency-wise)
            # only at whole-tranche completion, so coarse tranches stall
            # the projection loop at their boundaries; ~0.5-0.75 MB keeps
            # visibility ~1.5us apart vs ~4us per-tranche consumption.
            for lo, hi in ((128, 384), (384, 640), (640, 896), (896, 1280),
                           (1280, 1664), (1664, 2048)):
                nc.sync.dma_start(xt_sb[:, :, lo:hi], xT_r[:, :, lo:hi])
            # wq's consumer (P1) runs in the tail; ride after all of x^T
            nc.sync.dma_start(
                wq_sb[:], wq.ap().rearrange("(c p) n -> p c n", p=128)
            )
            wo_sb = pers.tile([128, QD_C, DM], MM_DT, tag="wo")
            nc.sync.dma_start(
                wo_sb[:], wo.ap().rearrange("(c p) n -> p c n", p=128)
            )
            # bqt's 8-byte descriptor rows make it a ~2.7us issue on the
            # Sync engine; it rides last (its consumer P1 runs ~30us in)
            nc.sync.dma_start(bq_sb[:], bqt.ap())

            qT_sb = pers.tile([128, QD_C, L], MM_DT, tag="qT")
            kv_sb = pers.tile([128, T_N, 2 * QD], MM_DT, tag="kv")
            mt_bd = pers.tile([128, QD_C, 128], MM_DT, tag="mt")
            wt_sb = pers.tile([128, QD_C, DM], MM_DT, tag="wt")

            nc.vector.memzero(mt_bd[:])

            def emit_p1(qc, tf):
                fsl = slice(tf * 512, (tf + 1) * 512)
                psq = ps.tile([128, 512], F32, tag="ps", name="psq")
                for dc in range(DM_C):
                    nc.tensor.matmul(
                        psq[:],
                        lhsT=wq_sb[:, dc, qc * 128:(qc + 1) * 128],
                        rhs=xt_sb[:, dc, fsl],
                        start=(dc == 0),
                        stop=(dc == DM_C - 1),
                    )
                nc.vector.tensor_scalar_add(
                    qT_sb[:, qc, fsl], psq[:], bq_sb[:, qc:qc + 1]
                )

            # -------- P2 + P3: K|V projections chasing the x^T loads, ------
            # with the M(hp) accumulations (MT_h += V_h^T K_h) chasing each
            # projected block so M closes right after the last block.  The
            # psm accumulators live across the whole phase, so they get
            # their own tag (slot rotation within a tag would otherwise
            # hand their banks to a later pskv allocation).
            psm = [
                ps.tile([128, 512], F32, tag="psm", bufs=2, name=f"psm{hp}")
                for hp in range(QD_C)
            ]
            for tf in range(T_F):
                for tl in range(4):
                    tn = 4 * tf + tl
                    tsl = slice(tn * 128, (tn + 1) * 128)
                    pskv = ps.tile([128, 512], F32, tag="ps", name="pskv")
                    for dc in range(DM_C):
                        nc.tensor.matmul(
                            pskv[:],
                            lhsT=xt_sb[:, dc, tsl],
                            rhs=wkv_sb[:, dc, :],
                            start=(dc == 0),
                            stop=(dc == DM_C - 1),
                        )
                    nc.vector.tensor_add(kv_sb[:, tn, :], pskv[:], bkv_bc[:])
                    for hp in range(QD_C):
                        nc.tensor.matmul(
                            psm[hp][:, 0:QD],
                            lhsT=kv_sb[:, tn, QD + hp * 128:QD + (hp + 1) * 128],
                            rhs=kv_sb[:, tn, 0:QD],
                            start=(tn == 0),
                            stop=(tn == T_N - 1),
                        )

            # -------- P4: W~ per head-pair ---------------------------------
            # The M -> scale -> W~ -> copy-back chain round-trips through
            # the DVE twice; P1 blocks (independent of M) are emitted
            # around it so the PE never stalls on those handoffs.
            for hp in range(QD_C):
                # mt_bd[:, hp] = blockdiag(M_h0^T, M_h1^T) * SCALE
                for j in range(2):
                    sl = slice(64 * j, 64 * j + 64)
                    nc.vector.tensor_scalar_mul(
                        mt_bd[sl, hp, sl],
                        psm[hp][sl, 128 * hp + 64 * j:128 * hp + 64 * j + 64],
                        SCALE,
                    )
            for qc in range(QD_C):
                emit_p1(qc, 0)
            for hp in range(QD_C):
                # W~ = mt_bd^T @ Wo_pair for this head-pair
                for oc in range(OUT_F):
                    psw = ps.tile([128, 512], F32, tag="ps", name="psw")
                    nc.tensor.matmul(
                        psw[:],
                        lhsT=mt_bd[:, hp, :],
                        rhs=wo_sb[:, hp, oc * 512:(oc + 1) * 512],
                        start=True,
                        stop=True,
                    )
                    nc.vector.tensor_copy(
                        wt_sb[:, hp, oc * 512:(oc + 1) * 512], psw[:]
                    )
            for qc in range(QD_C):
                emit_p1(qc, 1)

            # ------- tail: P1 + P5 interleaved per token block -> DRAM -----
            # Everything after P4 overlaps the output DMA.  Copy-backs are
            # split between DVE and ACT so neither engine paces the tail.
            for tf in range(T_F):
                if tf >= 2:
                    for qc in range(QD_C):
                        emit_p1(qc, tf)

                for tl in range(4):
                    tn = 4 * tf + tl
                    tsl = slice(tn * 128, (tn + 1) * 128)
                    p_tile = pstage.tile([128, DM], MM_DT, tag="pstage",
                                         name="p_tile")
                    for oc in range(OUT_F):
                        psp = ps.tile([128, 512], F32, tag="ps", name="psp")
                        for qc in range(QD_C):
                            nc.tensor.matmul(
                                psp[:],
                                lhsT=qT_sb[:, qc, tsl],
                                rhs=wt_sb[:, qc, oc * 512:(oc + 1) * 512],
                                start=(qc == 0),
                                stop=(qc == QD_C - 1),
                            )
                        osl = slice(oc * 512, (oc + 1) * 512)
                        if oc == 0:
                            nc.vector.tensor_copy(p_tile[:, osl], psp[:])
                        else:
                            nc.scalar.copy(p_tile[:, osl], psp[:])
                        if tn == T_N - 1:
                            # final block rides as two halves so the
                            # end-of-kernel flush only waits on 0.125 MB
                            nc.sync.dma_start(pout[tsl, osl], p_tile[:, osl])
                    if tn != T_N - 1:
                        nc.sync.dma_start(pout[tsl, :], p_tile[:])

    nc.compile()
    return nc


_PROGRAM = None


def _get_program():
    global _PROGRAM
    if _PROGRAM is None:
        _PROGRAM = build_program()
    return _PROGRAM


def _bf16(a):
    import ml_dtypes
    return np.ascontiguousarray(np.asarray(a, np.float32)).astype(
        ml_dtypes.bfloat16
    )


def kernel(x, Wq, bq, Wk, bk, Wv, bv, Wo, bo, _trace=False, _trace_kwargs=None):
    x = np.asarray(x, np.float32)
    Wq, bq = np.asarray(Wq, np.float32), np.asarray(bq, np.float32)
    Wk, bk = np.asarray(Wk, np.float32), np.asarray(bk, np.float32)
    Wv, bv = np.asarray(Wv, np.float32), np.asarray(bv, np.float32)
    Wo, bo = np.asarray(Wo, np.float32), np.asarray(bo, np.float32)

    nc = _get_program()

    xT = [_bf16(x[b].T) for b in range(B)]
    in_maps = []
    for c in range(NCORES):
        b, g = divmod(c, NCORES // B)
        sl = slice(g * QD, (g + 1) * QD)
        in_maps.append({
            "xT": xT[b],
            "wq": _bf16(Wq[:, sl]),
            "wkv": _bf16(np.concatenate([Wk[:, sl], Wv[:, sl]], axis=1)),
            "wo": _bf16(Wo[sl, :]),
            "bqt": np.ascontiguousarray(bq[sl].reshape(QD_C, 128).T),
            "bkvd": np.ascontiguousarray(
                np.concatenate([bk[sl], bv[sl]]).reshape(1, 2 * QD)
            ),
        })

    kw = {}
    if _trace:
        kw = dict(trace=True, trace_cores=list(range(NCORES)),
                  **(_trace_kwargs or {}))
    res = run_bass_kernel_spmd(nc, in_maps, list(range(NCORES)), **kw)

    out = np.empty((B, L, DM), np.float32)
    gpb = NCORES // B
    for b in range(B):
        acc = res.results[gpb * b]["pout"].astype(np.float32)
        for i in range(1, gpb):
            acc = acc + res.results[gpb * b + i]["pout"].astype(np.float32)
        out[b] = acc + bo
    kernel.last_results = res
    return out


# revision 24
# speedup vs baseline: 1.0332x; 1.0332x over previous
"""Multi-head attention (no softmax) Trainium2 Bass kernel, 8-core SPMD.

Reference computes out = ((X Wq + bq)(X Wk + bk)^T / 8)(X Wv + bv) Wo + bo
per head.  Since there is no softmax the per-head attention is linear:
    (Q_h K_h^T) V_h = Q_h (K_h^T V_h)
which collapses the 2048x2048 score matrix to a 64x64 per-head matrix.
Further folding (K_h^T V_h / 8) Wo_h into a data-dependent weight
W~ = stack_h(M_h Wo_h) makes the whole core computation:
    Q = X Wq_s,  KV = X [Wk_s|Wv_s],  MT_h = V_h^T K_h,
    W~_h = (MT_h^T / 8) Wo_h,  P = Q W~        (P is a partial of out)

Sharding: core c -> batch b = c // 4, head-group g = c % 4 (4 of 16 heads,
256 of 1024 projection columns).  No cross-device comms; the 4 partials per
batch are summed on the host (+ bo).

All matmul operands and DRAM traffic are bfloat16 (PE streams 1 row/cycle,
same as tf32, but SBUF/HBM bytes halve: ~10 MB/core total vs 21 MB at
fp32).  PSUM accumulation stays fp32; biases are added in fp32 on the
copy-backs; the output partial is written bf16 and the 4 partials per
batch are summed on the host in fp32 (measured end-to-end rel err ~4e-3
against the fp32 reference, vs a 2e-2 gate).

Schedule: dummy zero-matmuls warm the PE clock gate while the first DMAs
land; K|V projections chase the x^T token-block loads with the per-block
M accumulation (MT_h += V_h^T K_h) interleaved right behind each block
so M closes immediately after the last projection; W~ is the one sync
point (bracketed by Q^T projections, which are M-independent, to keep
the PE fed across its DVE handoffs); then Q^T and P = Q W~ interleave
per token block so the output DMA overlaps the remaining compute.

DMA notes (measured): each dma_start costs ~650ns of Sync-engine issue
time and per-core delivery tops out around 0.33 MB/us, so the input
rides as a handful of large transfers ordered to match the projection
loop's consumption (wkv halves interleaved with the first token blocks);
the PE streams chunk-by-chunk behind the DMA from ~10us and the kernel
is PE-bound from there (~61us of matmul cadence at 2.4 GHz).  Measured
best ~81.8us end-to-end per launch; device DVFS adds +-15% run-to-run.
"""

import numpy as np

import concourse.mybir as mybir
import concourse.tile as tile
from concourse import bacc
from concourse.bass_utils import run_bass_kernel_spmd

F32 = mybir.dt.float32
MM_DT = mybir.dt.bfloat16

B, L, DM = 2, 2048, 1024
QD = 256                 # per-core projection width (4 heads x 64)
HPC, HDIM = 4, 64
NCORES = 8
SCALE = 0.125            # 1 / sqrt(64)

DM_C = DM // 128         # 8 dmodel chunks
T_N = L // 128           # 16 token chunks (partition-sized)
T_F = L // 512           # 4 token chunks (free-dim sized)
QD_C = QD // 128         # 2 head-dim chunks / head-pairs
OUT_F = DM // 512        # 2 output free chunks


def build_program():
    nc = bacc.Bacc("TRN2", target_bir_lowering=False, debug=False)

    xT = nc.dram_tensor("xT", [DM, L], MM_DT, kind="ExternalInput")
    wq = nc.dram_tensor("wq", [DM, QD], MM_DT, kind="ExternalInput")
    wkv = nc.dram_tensor("wkv", [DM, 2 * QD], MM_DT, kind="ExternalInput")
    wo = nc.dram_tensor("wo", [QD, DM], MM_DT, kind="ExternalInput")
    bqt = nc.dram_tensor("bqt", [128, QD_C], F32, kind="ExternalInput")
    bkvd = nc.dram_tensor("bkvd", [1, 2 * QD], F32, kind="ExternalInput")
    pout = nc.dram_tensor("pout", [L, DM], MM_DT, kind="ExternalOutput")

    with tile.TileContext(nc) as tc:
        with (
            tc.tile_pool(name="persist", bufs=1) as pers,
            tc.tile_pool(name="pstage", bufs=4) as pstage,
            tc.tile_pool(name="psum", bufs=6, space="PSUM") as ps,
        ):
            # -------- PE warm-up: dummy matmuls with no DMA deps -----------
            # The HAM clock gate keeps an idle PE at 1.2 GHz; sustained
            # activity moves it to 2.4 GHz.  These zero-matmuls run while
            # the DMA descriptors are still being generated, so the first
            # real matmul executes at full rate.
            # Dummy matmuls bridge the PE from engine boot (~7us) until the
            # first projection's operands have landed (~12us); the HAM
            # up-clocks only after sustained activity, so the bridge also
            # pulls the 1.2->2.4 GHz transition several us earlier.
            N_WARM = 12
            warm_z = pers.tile([128, 512], MM_DT, tag="warmz")
            nc.vector.memzero(warm_z[:])
            warm_out = pers.tile([128, 512], F32, tag="warmout")
            pswarm = ps.tile([128, 512], F32, tag="ps", name="pswarm")
            for i in range(N_WARM):
                nc.tensor.matmul(
                    pswarm[:], lhsT=warm_z[:, 0:128], rhs=warm_z[:],
                    start=(i == 0), stop=(i == N_WARM - 1),
                )
            nc.vector.tensor_copy(warm_out[:], pswarm[:])

            # -------- wkv + x^T loads, interleaved per dmodel chunk ---------
            # The first K|V matmul needs only (wkv chunk 0, xt chunk 0 of
            # token block 0), so those bytes go on the queues first and the
            # PE starts a few us after the DMA engines spin up.  wq rides
            # after the first token block (its consumer P1 runs in the
            # tail); wo after all of x^T.
            wkv_sb = pers.tile([128, DM_C, 2 * QD], MM_DT, tag="wkv")
            wq_sb = pers.tile([128, DM_C, QD], MM_DT, tag="wq")
            xt_sb = pers.tile([128, DM_C, L], MM_DT, tag="xt")
            bq_sb = pers.tile([128, QD_C], F32, tag="bq")
            bkv_sb = pers.tile([1, 2 * QD], F32, tag="bkv1")
            bkv_bc = pers.tile([128, 2 * QD], F32, tag="bkvbc")
            # Each DMA instruction costs ~650ns of Sync-engine issue time,
            # so the input rides as a handful of large multi-chunk
            # transfers instead of per-chunk ones: all descriptors reach
            # the (16-way parallel) DMA hardware within ~3us of engine
            # boot and the transfer rate, not the issue rate, paces input.
            xT_r = xT.ap().rearrange("(c p) n -> p c n", p=128)
            wkv_r = wkv.ap().rearrange("(c p) n -> p c n", p=128)
            # wkv halves interleave with the first x^T tranches, and x^T
            # rides in arrival-ordered tranches sized to the P2 loop's
            # consumption (0.25 MB per 128-token block at ~0.13 MB/us,
            # under the ~0.2-0.33 MB/us delivery rate): the projection
            # loop's first matmuls start as soon as (wkv chunks 0-3, x
            # block 0) land and then stream chunk-by-chunk behind the DMA
            # (the open PSUM accumulation tolerates mid-group data waits),
            # so the PE never idles long enough for the HAM to down-clock.
            # x^T tranche boundaries follow the projection loop's measured
            # stall points: data is dependency-visible only at whole-
            # transfer completion, so the early tranches stay small (1-2
            # token blocks) and grow once the stream is ahead of the PE.
            nc.sync.dma_start(wkv_sb[:, 0:4, :], wkv_r[:, 0:4, :])
            nc.sync.dma_start(xt_sb[:, :, 0:128], xT_r[:, :, 0:128])
            nc.sync.dma_start(wkv_sb[:, 4:8, :], wkv_r[:, 4:8, :])
            nc.sync.dma_start(xt_sb[:, :, 128:256], xT_r[:, :, 128:256])
            nc.sync.dma_start(xt_sb[:, :, 256:384], xT_r[:, :, 256:384])
            nc.sync.dma_start(bkv_sb[:], bkvd.ap())
            nc.gpsimd.partition_broadcast(bkv_bc[:], bkv_sb[:])
            nc.sync.dma_start(xt_sb[:, :, 384:640], xT_r[:, :, 384:640])
            nc.sync.dma_start(xt_sb[:, :, 640:1024], xT_r[:, :, 640:1024])
            nc.sync.dma_start(xt_sb[:, :, 1024:1536], xT_r[:, :, 1024:1536])
            nc.sync.dma_start(xt_sb[:, :, 1536:L], xT_r[:, :, 1536:L])
            # wq's consumer (P1) runs in the tail; ride after all of x^T
            nc.sync.dma_start(
                wq_sb[:], wq.ap().rearrange("(c p) n -> p c n", p=128)
            )
            wo_sb = pers.tile([128, QD_C, DM], MM_DT, tag="wo")
            nc.sync.dma_start(
                wo_sb[:], wo.ap().rearrange("(c p) n -> p c n", p=128)
            )
            # bqt's 8-byte descriptor rows make it a ~2.7us issue on the
            # Sync engine; it rides last (its consumer P1 runs ~30us in)
            nc.sync.dma_start(bq_sb[:], bqt.ap())

            qT_sb = pers.tile([128, QD_C, L], MM_DT, tag="qT")
            kv_sb = pers.tile([128, T_N, 2 * QD], MM_DT, tag="kv")
            mt_bd = pers.tile([128, QD_C, 128], MM_DT, tag="mt")
            wt_sb = pers.tile([128, QD_C, DM], MM_DT, tag="wt")

            nc.vector.memzero(mt_bd[:])

            def emit_p1(qc, tf):
                fsl = slice(tf * 512, (tf + 1) * 512)
                psq = ps.tile([128, 512], F32, tag="ps", name="psq")
                for dc in range(DM_C):
                    nc.tensor.matmul(
                        psq[:],
                        lhsT=wq_sb[:, dc, qc * 128:(qc + 1) * 128],
                        rhs=xt_sb[:, dc, fsl],
                        start=(dc == 0),
                        stop=(dc == DM_C - 1),
                    )
                nc.vector.tensor_scalar_add(
                    qT_sb[:, qc, fsl], psq[:], bq_sb[:, qc:qc + 1]
                )

            # -------- P2 + P3: K|V projections chasing the x^T loads, ------
            # with the M(hp) accumulations (MT_h += V_h^T K_h) chasing each
            # projected block so M closes right after the last block.  The
            # psm accumulators live across the whole phase, so they get
            # their own tag (slot rotation within a tag would otherwise
            # hand their banks to a later pskv allocation).
            psm = [
                ps.tile([128, 512], F32, tag="psm", bufs=2, name=f"psm{hp}")
                for hp in range(QD_C)
            ]
            for tf in range(T_F):
                for tl in range(4):
                    tn = 4 * tf + tl
                    tsl = slice(tn * 128, (tn + 1) * 128)
                    pskv = ps.tile([128, 512], F32, tag="ps", name="pskv")
                    for dc in range(DM_C):
                        nc.tensor.matmul(
                            pskv[:],
                            lhsT=xt_sb[:, dc, tsl],
                            rhs=wkv_sb[:, dc, :],
                            start=(dc == 0),
                            stop=(dc == DM_C - 1),
                        )
                    nc.vector.tensor_add(kv_sb[:, tn, :], pskv[:], bkv_bc[:])
                    for hp in range(QD_C):
                        nc.tensor.matmul(
                            psm[hp][:, 0:QD],
                            lhsT=kv_sb[:, tn, QD + hp * 128:QD + (hp + 1) * 128],
                            rhs=kv_sb[:, tn, 0:QD],
                            start=(tn == 0),
                            stop=(tn == T_N - 1),
                        )

            # -------- P4: W~ per head-pair ---------------------------------
            # The M -> scale -> W~ -> copy-back chain round-trips through
            # the DVE twice; P1 blocks (independent of M) are emitted
            # around it so the PE never stalls on those handoffs.
            for hp in range(QD_C):
                # mt_bd[:, hp] = blockdiag(M_h0^T, M_h1^T) * SCALE
                for j in range(2):
                    sl = slice(64 * j, 64 * j + 64)
                    nc.vector.tensor_scalar_mul(
                        mt_bd[sl, hp, sl],
                        psm[hp][sl, 128 * hp + 64 * j:128 * hp + 64 * j + 64],
                        SCALE,
                    )
            for qc in range(QD_C):
                emit_p1(qc, 0)
            for hp in range(QD_C):
                # W~ = mt_bd^T @ Wo_pair for this head-pair
                for oc in range(OUT_F):
                    psw = ps.tile([128, 512], F32, tag="ps", name="psw")
                    nc.tensor.matmul(
                        psw[:],
                        lhsT=mt_bd[:, hp, :],
                        rhs=wo_sb[:, hp, oc * 512:(oc + 1) * 512],
                        start=True,
                        stop=True,
                    )
                    nc.vector.tensor_copy(
                        wt_sb[:, hp, oc * 512:(oc + 1) * 512], psw[:]
                    )
            for qc in range(QD_C):
                emit_p1(qc, 1)

            # ------- tail: P1 + P5 interleaved per token block -> DRAM -----
            # Everything after P4 overlaps the output DMA.  Copy-backs are
            # split between DVE and ACT so neither engine paces the tail.
            for tf in range(T_F):
                if tf >= 2:
                    for qc in range(QD_C):
                        emit_p1(qc, tf)

                for tl in range(4):
                    tn = 4 * tf + tl
                    tsl = slice(tn * 128, (tn + 1) * 128)
                    p_tile = pstage.tile([128, DM], MM_DT, tag="pstage",
                                         name="p_tile")
                    for oc in range(OUT_F):
                        psp = ps.tile([128, 512], F32, tag="ps", name="psp")
                        for qc in range(QD_C):
                            nc.tensor.matmul(
                                psp[:],
                                lhsT=qT_sb[:, qc, tsl],
                                rhs=wt_sb[:, qc, oc * 512:(oc + 1) * 512],
                                start=(qc == 0),
                                stop=(qc == QD_C - 1),
                            )
                        osl = slice(oc * 512, (oc + 1) * 512)
                        if oc == 0:
                            nc.vector.tensor_copy(p_tile[:, osl], psp[:])
                        else:
                            nc.scalar.copy(p_tile[:, osl], psp[:])
                    nc.sync.dma_start(pout[tsl, :], p_tile[:])

    nc.compile()
    return nc


_PROGRAM = None


def _get_program():
    global _PROGRAM
    if _PROGRAM is None:
        _PROGRAM = build_program()
    return _PROGRAM


def _bf16(a):
    import ml_dtypes
    return np.ascontiguousarray(np.asarray(a, np.float32)).astype(
        ml_dtypes.bfloat16
    )


def kernel(x, Wq, bq, Wk, bk, Wv, bv, Wo, bo, _trace=False, _trace_kwargs=None):
    x = np.asarray(x, np.float32)
    Wq, bq = np.asarray(Wq, np.float32), np.asarray(bq, np.float32)
    Wk, bk = np.asarray(Wk, np.float32), np.asarray(bk, np.float32)
    Wv, bv = np.asarray(Wv, np.float32), np.asarray(bv, np.float32)
    Wo, bo = np.asarray(Wo, np.float32), np.asarray(bo, np.float32)

    nc = _get_program()

    xT = [_bf16(x[b].T) for b in range(B)]
    in_maps = []
    for c in range(NCORES):
        b, g = divmod(c, NCORES // B)
        sl = slice(g * QD, (g + 1) * QD)
        in_maps.append({
            "xT": xT[b],
            "wq": _bf16(Wq[:, sl]),
            "wkv": _bf16(np.concatenate([Wk[:, sl], Wv[:, sl]], axis=1)),
            "wo": _bf16(Wo[sl, :]),
            "bqt": np.ascontiguousarray(bq[sl].reshape(QD_C, 128).T),
            "bkvd": np.ascontiguousarray(
                np.concatenate([bk[sl], bv[sl]]).reshape(1, 2 * QD)
            ),
        })

    kw = {}
    if _trace:
        kw = dict(trace=True, trace_cores=list(range(NCORES)),
                  **(_trace_kwargs or {}))
    res = run_bass_kernel_spmd(nc, in_maps, list(range(NCORES)), **kw)

    out = np.empty((B, L, DM), np.float32)
    gpb = NCORES // B
    for b in range(B):
        acc = res.results[gpb * b]["pout"].astype(np.float32)
        for i in range(1, gpb):
            acc = acc + res.results[gpb * b + i]["pout"].astype(np.float32)
        out[b] = acc + bo
    kernel.last_results = res
    return out
